# revision 29
# baseline (speedup 1.0000x reference)
"""Trainium2 Bass kernel for nn_Attention_48498770706573.

Fused QKV-projection + masked softmax attention, sharded over 8 NeuronCores:
data-parallel over batch (B=2), tensor-parallel over heads (16 -> 4 per
core). Each core computes its (batch, 4-head) shard end to end; the host
only slices/transposes/bf16-casts inputs (no arithmetic beyond dtype
rounding) and concatenates the disjoint output shards.

The kernel is ACT(exp)-bound: 128 exps of [128,1024] ~= 130us of Scalar
engine time. The structure maximizes ACT occupancy:
  - inputs arrive pre-cast bf16 in partition-major chunk layouts so each
    k/q/v chunk is ONE dma_start (128 descriptors x 8KB) -- the DMA
    queue issues in ~1us instead of ~5us per chunk,
  - a minimal pre-phase (k-proj chunk0 + q-proj chunk0) so the first
    exp fires ~10us in,
  - the remaining projections (k-proj chunks 1-3, all of v-proj) are
    interleaved into chunk 0's attention iterations through the psum
    slots that are free at that point (pvps before PV starts, the
    rotating rsps utility slot after),
  - scores S^T[nk, nq] as bf16 matmuls, two heads row-packed via
    base_partition (concurrent in the PE array), exp on ACT straight
    out of PSUM (1/32 scale folded in), bool mask cast u8->bf16 via
    SWDGE cast-DMA and applied with one broadcast DVE multiply,
  - PV with p^T bf16 moving, heads col-packed (concurrent), row-sums
    via col-packed ones matmuls; the reciprocal of the row-sum runs on
    DVE (reciprocal_approx_fast, f32) so chunk tails never block the
    ACT queue, and the V-bias is added on the OUTPUT (out = pv/rs + bv
    exactly -- the bias passes through the softmax normalization), so
    the tail needs only one rank-1 broadcast matmul per pair,
  - the last LAG PV steps + tail of each chunk are deferred into the
    next chunk's first iterations; next-chunk q-projection is emitted
    near the end of the current chunk (split into 4-matmul parts for
    steady chunks); outputs written bf16 on the SWDGE queue to keep
    the input queue clean.
"""

import os

import numpy as np

import concourse.bacc as bacc
import concourse.mybir as mybir
import concourse.tile as tile
from concourse.bass_utils import run_bass_kernel_spmd

B, NQ, NK, D, H = 2, 2048, 2048, 1024, 16
DH = D // H  # 64
N_CORES = 8
HPC = H // (N_CORES // B)  # heads per core = 4
JW = HPC * DH  # per-core projection width = 256
NKT = NK // 128  # 16 nk tiles
NCH = 4  # nq chunks
CHW = NQ // NCH  # 512
DT = 8  # contraction d-tiles

f32 = mybir.dt.float32
f32r = mybir.dt.float32r
bf16 = mybir.dt.bfloat16
u8 = mybir.dt.uint8


def _build():
    nc = bacc.Bacc(
        "TRN2", target_bir_lowering=False, debug=False, num_devices=N_CORES
    )

    # x tensors in partition-major chunk layout: X[p, ch, d, n] =
    # x[ch*CHW + n, d*128 + p] -- one contiguous 8KB run per partition
    # per chunk, so a chunk is a single 128-descriptor dma_start.
    qTd = nc.dram_tensor("qT", [128, NCH, DT, CHW], bf16, kind="ExternalInput")
    kTd = nc.dram_tensor("kT", [128, NCH, DT, CHW], bf16, kind="ExternalInput")
    vTd = nc.dram_tensor("vT", [128, NCH, DT, CHW], bf16, kind="ExternalInput")
    # mask: M[p, t, n] = mask[n, t*128 + p]
    maskd = nc.dram_tensor("maskT", [128, NKT, NQ], u8, kind="ExternalInput")
    # weights: W[p, d, j] = w[d*128 + p, j]
    wqd = nc.dram_tensor("wqT", [128, DT, JW], bf16, kind="ExternalInput")
    wkd = nc.dram_tensor("wkT", [128, DT, JW], bf16, kind="ExternalInput")
    wvd = nc.dram_tensor("wvT", [128, DT, JW], bf16, kind="ExternalInput")
    bqd = nc.dram_tensor("bq", [128, 2], f32, kind="ExternalInput")
    bkd = nc.dram_tensor("bk", [128, 2], f32, kind="ExternalInput")
    # bvp[p, pair] = bv[128*pair + p]
    bvpd = nc.dram_tensor("bvp", [128, 2], f32, kind="ExternalInput")
    # ones2[64p, 0:64] = 1, ones2[64p + 32, 64:128] = 1
    onesd = nc.dram_tensor("ones2", [128, 128], f32r, kind="ExternalInput")
    onespd = nc.dram_tensor("onesp", [128, 32], bf16, kind="ExternalInput")
    o = nc.dram_tensor("o", [2 * 128, NQ], bf16, kind="ExternalOutput")

    with tile.TileContext(nc) as tc:
        with (
            tc.tile_pool(name="consts", bufs=1) as consts,
            tc.tile_pool(name="kst", bufs=3) as kst,
            tc.tile_pool(name="qst", bufs=2) as qst,
            tc.tile_pool(name="vst", bufs=3) as vst,
            tc.tile_pool(name="m8pool", bufs=8) as m8pool,
            tc.tile_pool(name="mbpool", bufs=7) as mbpool,
            tc.tile_pool(name="projout", bufs=1) as projout,
            tc.tile_pool(name="ppool", bufs=16) as ppool,
            tc.tile_pool(name="rspool", bufs=1) as rspool,
            tc.tile_pool(name="outsb", bufs=2) as outsb,
            tc.tile_pool(name="sps", bufs=2, space="PSUM") as sps,
            tc.tile_pool(name="pvps", bufs=2, space="PSUM") as pvps,
            tc.tile_pool(name="rsps", bufs=2, space="PSUM") as rsps,
        ):
            def dma_w(name, dram):
                t = consts.tile([128, DT, JW], bf16, tag=f"w{name}", name="w")
                nc.sync.dma_start(t, dram[:])
                return t

            def dma_x(src, ch, pool, tag, split=False):
                x = pool.tile([128, DT, CHW], bf16, tag=tag, name=tag)
                if split:
                    # two halves so the d 0-3 matmuls can start earlier
                    nc.sync.dma_start(x[:, 0:4], src[:, ch, 0:4])
                    nc.sync.dma_start(x[:, 4:8], src[:, ch, 4:8])
                else:
                    nc.sync.dma_start(x, src[:, ch])
                return x

            def dma_m(g):
                """Mask tiles 2g, 2g+1. On the sync queue, placed in
                consumption order: the DMA engines drain mostly FIFO, so a
                parallel-queue mask would overtake the critical k/q path."""
                mt8 = m8pool.tile([128, 2, NQ], u8, tag="m8", name="m8")
                nc.sync.dma_start(mt8, maskd[:, 2 * g : 2 * g + 2, :])
                return mt8

            # ---- input DMAs, emitted in consumption order ----
            bq_sb = consts.tile([128, 2], f32, tag="bq")
            nc.sync.dma_start(bq_sb, bqd[:])
            bk_sb = consts.tile([128, 2], f32, tag="bk")
            nc.sync.dma_start(bk_sb, bkd[:])
            onesp_sb = consts.tile([128, 32], bf16, tag="onesp")
            nc.sync.dma_start(onesp_sb, onespd[:])
            w_k = dma_w("k", wkd)
            k_x = {0: dma_x(kTd, 0, kst, "kx", split=True)}
            w_q = dma_w("q", wqd)
            q_x = {0: dma_x(qTd, 0, qst, "qx", split=True)}
            m8 = [dma_m(0)]
            k_x[1] = dma_x(kTd, 1, kst, "kx")
            m8.append(dma_m(1))
            w_v = dma_w("v", wvd)
            v_x = {0: dma_x(vTd, 0, vst, "vx")}
            m8.append(dma_m(2))
            v_x[1] = dma_x(vTd, 1, vst, "vx")
            k_x[2] = dma_x(kTd, 2, kst, "kx")
            m8.append(dma_m(3))
            v_x[2] = dma_x(vTd, 2, vst, "vx")
            m8.append(dma_m(4))
            m8.append(dma_m(5))
            k_x[3] = dma_x(kTd, 3, kst, "kx")
            v_x[3] = dma_x(vTd, 3, vst, "vx")
            m8.append(dma_m(6))
            m8.append(dma_m(7))
            bvp_sb = consts.tile([128, 2], f32, tag="bvp")
            nc.sync.dma_start(bvp_sb, bvpd[:])
            ones_sb = consts.tile([128, 128], f32r, tag="ones")
            nc.sync.dma_start(ones_sb, onesd[:])
            q_x[1] = dma_x(qTd, 1, qst, "qx")
            q_x[2] = dma_x(qTd, 2, qst, "qx")
            q_x[3] = dma_x(qTd, 3, qst, "qx")

            # ---- projection outputs (split per chunk for clean deps) ----
            ktTs = [
                projout.tile([128, 2, CHW], bf16, tag=f"ktT{c}", name="ktT")
                for c in range(NCH)
            ]
            qtTs = [
                projout.tile([128, 2, CHW], bf16, tag=f"qtT{c}", name="qtT")
                for c in range(NCH)
            ]
            # vts[g][:, a, :] = vt for nk-tile 2g+a
            vts = [
                projout.tile([128, 2, JW], bf16, tag=f"vt{g}", name="vt")
                for g in range(NKT // 2)
            ]

            def proj_qk_sps(w, xs, dst, bias):
                """q/k projection chunk through one 2-bank sps tile:
                m0 -> cols 0:CHW, m1 -> cols CHW:2CHW."""
                ps = sps.tile([128, 2 * CHW], f32, tag="s", name="pps")
                for d in range(DT):
                    for m in range(2):
                        nc.tensor.matmul(
                            ps[:, m * CHW : (m + 1) * CHW],
                            w[:, d, m * 128 : (m + 1) * 128],
                            xs[:, d],
                            start=(d == 0),
                            stop=(d == DT - 1),
                        )
                for m in range(2):
                    nc.vector.tensor_scalar_add(
                        dst[:, m, :],
                        ps[:, m * CHW : (m + 1) * CHW],
                        bias[:, m : m + 1],
                    )

            def proj_qk_m(w, xs, dst, bias, m, pool, nm):
                """One m-half of a q/k projection chunk through a single
                [128, CHW] psum tile from `pool`."""
                ps = pool.tile([128, CHW], f32, tag=nm, name="pps")
                for d in range(DT):
                    nc.tensor.matmul(
                        ps,
                        w[:, d, m * 128 : (m + 1) * 128],
                        xs[:, d],
                        start=(d == 0),
                        stop=(d == DT - 1),
                    )
                nc.vector.tensor_scalar_add(
                    dst[:, m, :], ps, bias[:, m : m + 1]
                )

            def proj_qk_m_part(w, xs, dst, bias, m, part, state):
                """proj_qk_m split into two 4-matmul halves so a steady
                chunk's next-q projection never spikes one iteration's PE
                load."""
                if part == 0:
                    state[m] = rsps.tile([128, CHW], f32, tag="rspst", name="pps")
                ps = state[m]
                for d in (range(0, 4) if part == 0 else range(4, DT)):
                    nc.tensor.matmul(
                        ps,
                        w[:, d, m * 128 : (m + 1) * 128],
                        xs[:, d],
                        start=(d == 0),
                        stop=(d == DT - 1),
                    )
                if part == 1:
                    nc.vector.tensor_scalar_add(
                        dst[:, m, :], ps, bias[:, m : m + 1]
                    )

            def vproj_pair(g):
                """vt for nk-tiles 2g, 2g+1 through the rsps utility slot."""
                ps = rsps.tile([128, CHW], f32, tag="rspst", name="vps")
                ps2 = ps.rearrange("p (a j) -> p a j", a=2)
                for d in range(DT):
                    for a in range(2):
                        t = 2 * g + a
                        ch, nn_ = divmod(t, 4)
                        # start=True clears the WHOLE bank (all 512 cols) in
                        # the written partitions, so only the very first
                        # matmul of the packed pair may set it; the a=1
                        # group's first matmul overwrites where has_written
                        # is clear (bank-wide clear reset its bits too).
                        nc.tensor.matmul(
                            ps2[:, a],
                            v_x[ch][:, d, nn_ * 128 : (nn_ + 1) * 128],
                            w_v[:, d, :],
                            start=(d == 0 and a == 0),
                            stop=(d == DT - 1),
                        )
                nc.vector.tensor_copy(vts[g][:], ps2)

            # ---- attention ----
            def scores_group(pair, t, ch, p_tiles):
                sp = sps.tile([128, 2 * CHW], f32, tag="s", name="sp")
                kc, tt = ktTs[t // 4], t % 4
                for hh in range(2):
                    nc.tensor.matmul(
                        sp[:, hh * CHW : (hh + 1) * CHW],
                        kc[
                            64 * hh : 64 * (hh + 1),
                            pair,
                            tt * 128 : (tt + 1) * 128,
                        ],
                        qtTs[ch][64 * hh : 64 * (hh + 1), pair, :],
                        start=True,
                        stop=True,
                    )
                p = ppool.tile([128, 2 * CHW], bf16, tag="p", name="p")
                nc.scalar.activation(
                    out=p,
                    in_=sp,
                    func=mybir.ActivationFunctionType.Exp,
                    scale=1.0 / 32.0,
                )
                if pair == 0:
                    mb = mbpool.tile([128, CHW], bf16, tag="mb", name="mb")
                    # SWDGE cast DMA u8 -> bf16 (frees GpSimd compute)
                    nc.gpsimd.dma_start(
                        mb, m8[t // 2][:, t % 2, ch * CHW : (ch + 1) * CHW]
                    )
                    p_tiles[("mb", t)] = mb
                p_tiles[(pair, t)] = p

            def mask_mul(t, p_tiles):
                # deferred from scores_group so utility-phase drains don't
                # queue behind a burst of muls on the in-order DVE queue
                mb = p_tiles[("mb", t)]
                for pair in range(2):
                    p3 = p_tiles[(pair, t)].rearrange("p (h c) -> p h c", h=2)
                    nc.vector.tensor_mul(
                        p3,
                        p3,
                        mb.rearrange("p (a c) -> p a c", a=1).to_broadcast(
                            (128, 2, CHW)
                        ),
                    )

            def pv_t(t, p_tiles, pv_ps, rs_ps):
                st, sp_ = t == 0, t == NKT - 1
                g, a = divmod(t, 2)
                for pair in range(2):
                    p = p_tiles[(pair, t)]
                    for hh in range(2):
                        nc.tensor.matmul(
                            pv_ps[pair][64 * hh : 64 * (hh + 1), :],
                            vts[g][
                                :,
                                a,
                                128 * pair + 64 * hh : 128 * pair
                                + 64 * (hh + 1),
                            ],
                            p[:, hh * CHW : (hh + 1) * CHW],
                            start=st,
                            stop=sp_,
                            tile_position=(0, 64 * hh),
                        )
                for pair in range(2):
                    p = p_tiles[(pair, t)]
                    for hh in range(2):
                        hg = 2 * pair + hh
                        nc.tensor.matmul(
                            rs_ps[32 * hg : 32 * hg + 32, :],
                            onesp_sb[:, 0:32],
                            p[:, hh * CHW : (hh + 1) * CHW],
                            start=st,
                            stop=sp_,
                            tile_position=(0, 32 * hg),
                        )

            def chunk_tail(cs, pv_ps, rs_ps):
                # rowsum -> +eps -> reciprocal (all 128 rows valid: the M=32
                # rowsum matmuls wrote 32 identical rows per head). The whole
                # tail runs on DVE + GpSimd: no PE matmuls and no ACT ops, so
                # it never blocks next-chunk scores or the exp stream.
                rs_sb = rspool.tile([128, CHW], f32, tag="rssb", name="rssb")
                nc.vector.tensor_scalar_add(rs_sb, rs_ps, 1e-6)
                rc_sb = rspool.tile([128, CHW], f32, tag="rcsb", name="rcsb")
                nc.vector.reciprocal_approx_fast(out=rc_sb, in_=rs_sb)
                # bit-identical copy to f32r so the rank-1 broadcast matmul
                # streams fast (plain-f32 moving is 4x slower on the PE)
                rc_r = rspool.tile([128, CHW], f32r, tag="rcr", name="rcr")
                nc.vector.tensor_copy(rc_r, rc_sb)
                for pair in range(2):
                    # rb = broadcast of 1/(rs+eps) to the pair's 128 rows
                    rb = rsps.tile([128, CHW], f32, tag="rspst", name="rb")
                    nc.tensor.matmul(
                        rb,
                        ones_sb[64 * pair : 64 * (pair + 1), :],
                        rc_r[64 * pair : 64 * (pair + 1), :],
                        start=True,
                        stop=True,
                    )
                    rb_sb = outsb.tile([128, CHW], f32, tag="rbsb", name="rbsb")
                    nc.vector.tensor_copy(rb_sb, rb)
                    osb = outsb.tile([128, CHW], bf16, tag="o", name="osb")
                    nc.vector.tensor_mul(osb, pv_ps[pair], rb_sb)
                    # out = pv/rs + bv  (the V-bias passes straight through
                    # the softmax normalization)
                    nc.vector.tensor_scalar_add(
                        osb, osb, bvp_sb[:, pair : pair + 1]
                    )
                    nc.gpsimd.dma_start(
                        o[128 * pair : 128 * (pair + 1), cs], osb
                    )

            def new_pv_tiles():
                pv_ps = [
                    pvps.tile([128, CHW], f32, tag="pvpst", name=f"pv{i}")
                    for i in range(2)
                ]
                rs_ps = rsps.tile([128, CHW], f32, tag="rspst", name="rsps_t")
                return pv_ps, rs_ps

            # ---- pre-phase: k-proj chunks 0+1, q-proj chunk0 ----
            proj_qk_sps(w_k, k_x[0], ktTs[0], bk_sb)
            proj_qk_sps(w_q, q_x[0], qtTs[0], bq_sb)

            # chunk 0 utility-phase schedule (kc1 through the pvps slots
            # before PV claims them; everything else through the rsps
            # rotating slot). Keyed by iteration; placed so each phase's
            # inputs (DMA) land just before and its output is ready just
            # before its first consumer. Emitted AFTER that iteration's
            # scores so the exps fire first.
            util0 = {
                1: [lambda: proj_qk_m(w_k, k_x[1], ktTs[1], bk_sb, 0, pvps, "pvpst")],
                2: [lambda: proj_qk_m(w_k, k_x[1], ktTs[1], bk_sb, 1, pvps, "pvpst")],
                3: [lambda: vproj_pair(0)],
                4: [lambda: vproj_pair(1)],
                5: [lambda: vproj_pair(2)],
                6: [lambda: proj_qk_m(w_k, k_x[2], ktTs[2], bk_sb, 0, rsps, "rspst")],
                7: [lambda: proj_qk_m(w_k, k_x[2], ktTs[2], bk_sb, 1, rsps, "rspst")],
                8: [lambda: vproj_pair(3)],
                9: [lambda: vproj_pair(4)],
                10: [lambda: proj_qk_m(w_k, k_x[3], ktTs[3], bk_sb, 0, rsps, "rspst")],
                11: [lambda: proj_qk_m(w_k, k_x[3], ktTs[3], bk_sb, 1, rsps, "rspst")],
                12: [lambda: vproj_pair(5)],
                13: [lambda: vproj_pair(6)],
                14: [lambda: vproj_pair(7)],
            }

            # All chunks fully interleaved. The last LAG PV steps + tail of
            # chunk c are carried into chunk c+1's first iterations so the
            # ACT stream never waits for a PE drain at a chunk boundary.
            # Chunk c+1's q-projection runs mid-chunk-c through the rsps
            # slot (free after the carried tail releases it).
            LAG = 3
            pending = []
            for ch in range(NCH):
                cs = slice(ch * CHW, (ch + 1) * CHW)
                p_tiles = {}
                qp_state = {}
                pv_ps, rs_ps = None, None
                for t in range(NKT):
                    if (ch == 0 and t == 3) or (ch > 0 and t == 0):
                        pv_ps, rs_ps = new_pv_tiles()
                    for pair in range(2):
                        scores_group(pair, t, ch, p_tiles)
                    if ch == 0 and t in util0:
                        for fn in util0[t]:
                            fn()
                    if pending:
                        pending.pop(0)()
                    if t >= 2:
                        mask_mul(t - 2, p_tiles)
                    if t >= LAG:
                        pv_t(t - LAG, p_tiles, pv_ps, rs_ps)
                    if ch + 1 < NCH:
                        if ch == 0:
                            if t == 14:
                                proj_qk_m(w_q, q_x[1], qtTs[1], bq_sb, 0,
                                          rsps, "rspst")
                            elif t == 15:
                                proj_qk_m(w_q, q_x[1], qtTs[1], bq_sb, 1,
                                          rsps, "rspst")
                        elif 11 <= t <= 14:
                            m, part = divmod(t - 11, 2)
                            proj_qk_m_part(
                                w_q, q_x[ch + 1], qtTs[ch + 1], bq_sb,
                                m, part, qp_state,
                            )
                # muls for the last two tiles before their carried PVs
                mask_mul(NKT - 2, p_tiles)
                mask_mul(NKT - 1, p_tiles)

                def _carry(tt, p_tiles=p_tiles, pv_ps=pv_ps, rs_ps=rs_ps):
                    pv_t(tt, p_tiles, pv_ps, rs_ps)

                def _tail(cs=cs, pv_ps=pv_ps, rs_ps=rs_ps):
                    chunk_tail(cs, pv_ps, rs_ps)

                pending = [
                    lambda f=_carry: f(NKT - 3),
                    lambda f=_carry: f(NKT - 2),
                    lambda f=_carry, g=_tail: (f(NKT - 1), g()),
                ]
            for fn in pending:
                fn()

    nc.compile()
    return nc


_NC = None


def _get_nc():
    global _NC
    if _NC is None:
        _NC = _build()
    return _NC


def _shard(inputs):
    import ml_dtypes

    bfl = ml_dtypes.bfloat16
    q, k, v = inputs["q"], inputs["k"], inputs["v"]
    mask = inputs["mask"]
    Wq, bq, Wk, bk, Wv, bv = (
        inputs[n] for n in ("Wq", "bq", "Wk", "bk", "Wv", "bv")
    )

    def xfmt(x):
        # [N, D] -> [128, NCH, DT, CHW]: X[p, ch, d, n] = x[ch*CHW+n, 128d+p]
        return np.ascontiguousarray(
            np.asarray(x)
            .reshape(NCH, CHW, DT, 128)
            .transpose(3, 0, 2, 1)
            .astype(bfl)
        )

    def wfmt(w, j0):
        # [D, D] -> [128, DT, JW]: W[p, d, j] = w[j0+j, 128d+p]
        return np.ascontiguousarray(
            np.asarray(w)[j0 : j0 + JW, :].T.reshape(DT, 128, JW)
            .transpose(1, 0, 2)
            .astype(bfl)
        )

    qX = [xfmt(q[b]) for b in range(B)]
    kX = [xfmt(k[b]) for b in range(B)]
    vX = [xfmt(v[b]) for b in range(B)]
    # mask [NQ, NK] bool -> [128, NKT, NQ] u8: M[p, t, n] = mask[n, 128t+p]
    mX = [
        np.ascontiguousarray(
            np.asarray(mask[b]).T.reshape(NKT, 128, NQ).transpose(1, 0, 2)
        ).view(np.uint8)
        for b in range(B)
    ]
    onesp = np.ones((128, 32), bfl)
    ones2 = np.zeros((128, 128), np.float32)
    for p in range(2):
        ones2[64 * p, 0:64] = 1.0
        ones2[64 * p + 32, 64:128] = 1.0
    in_maps = []
    for c in range(N_CORES):
        b, jg = divmod(c, N_CORES // B)
        j0 = jg * JW
        bvp = np.ascontiguousarray(
            np.asarray(bv, np.float32)[j0 : j0 + JW].reshape(2, 128).T
        )
        in_maps.append(
            {
                "qT": qX[b],
                "kT": kX[b],
                "vT": vX[b],
                "maskT": mX[b],
                "wqT": wfmt(Wq, j0),
                "wkT": wfmt(Wk, j0),
                "wvT": wfmt(Wv, j0),
                "bq": np.ascontiguousarray(
                    np.asarray(bq, np.float32)[j0 : j0 + JW].reshape(2, 128).T
                ),
                "bk": np.ascontiguousarray(
                    np.asarray(bk, np.float32)[j0 : j0 + JW].reshape(2, 128).T
                ),
                "bvp": bvp,
                "ones2": ones2,
                "onesp": onesp,
            }
        )
    return in_maps


LAST_RESULT = None


def kernel(**inputs) -> np.ndarray:
    global LAST_RESULT
    nc = _get_nc()
    in_maps = _shard(inputs)
    trace = bool(int(os.environ.get("KTRACE", "0")))
    res = run_bass_kernel_spmd(
        nc,
        in_maps,
        core_ids=list(range(N_CORES)),
        trace=trace,
        trace_cores=[0] if trace else None,
    )
    LAST_RESULT = res
    out = np.empty((B, NQ, D), np.float32)
    for c in range(N_CORES):
        b, jg = divmod(c, N_CORES // B)
        j0 = jg * JW
        oc = res.results[c]["o"].astype(np.float32)  # [256, NQ] pair-major
        out[b, :, j0 : j0 + JW] = (
            oc.reshape(2, 2, DH, NQ).transpose(3, 0, 1, 2).reshape(NQ, JW)
        )
    return out


if __name__ == "__main__":
    if os.environ.get("KBUILD_ONLY"):
        import tempfile

        from concourse.bass_utils import compile_bass_kernel

        nc = _build()
        with tempfile.TemporaryDirectory() as td:
            compile_bass_kernel(nc, td)
        print("BUILD+COMPILE OK")


# revision 30
# speedup vs baseline: 1.0063x; 1.0063x over previous
"""Trainium2 Bass kernel for nn_Attention_48498770706573.

Fused QKV-projection + masked softmax attention, sharded over 8 NeuronCores:
data-parallel over batch (B=2), tensor-parallel over heads (16 -> 4 per
core). Each core computes its (batch, 4-head) shard end to end; the host
only slices/transposes/bf16-casts inputs (no arithmetic beyond dtype
rounding) and concatenates the disjoint output shards.

The kernel is ACT(exp)-bound: 128 exps of [128,1024] ~= 130us of Scalar
engine time. The structure maximizes ACT occupancy:
  - inputs arrive pre-cast bf16 in partition-major chunk layouts so each
    k/q/v chunk is ONE dma_start (128 descriptors x 8KB) -- the DMA
    queue issues in ~1us instead of ~5us per chunk,
  - a minimal pre-phase (k-proj chunk0 + q-proj chunk0) so the first
    exp fires ~10us in,
  - the remaining projections (k-proj chunks 1-3, all of v-proj) are
    interleaved into chunk 0's attention iterations through the psum
    slots that are free at that point (pvps before PV starts, the
    rotating rsps utility slot after),
  - scores S^T[nk, nq] as bf16 matmuls, two heads row-packed via
    base_partition (concurrent in the PE array), exp on ACT straight
    out of PSUM (1/32 scale folded in), bool mask cast u8->bf16 via
    SWDGE cast-DMA and applied with one broadcast DVE multiply,
  - PV with p^T bf16 moving, heads col-packed (concurrent), row-sums
    via col-packed ones matmuls; the reciprocal of the row-sum runs on
    DVE (reciprocal_approx_fast, f32) so chunk tails never block the
    ACT queue, and the V-bias is added on the OUTPUT (out = pv/rs + bv
    exactly -- the bias passes through the softmax normalization), so
    the tail needs only one rank-1 broadcast matmul per pair,
  - the last LAG PV steps + tail of each chunk are deferred into the
    next chunk's first iterations; next-chunk q-projection is emitted
    near the end of the current chunk (split into 4-matmul parts for
    steady chunks); outputs written bf16 on the SWDGE queue to keep
    the input queue clean.
"""

import os

import numpy as np

import concourse.bacc as bacc
import concourse.mybir as mybir
import concourse.tile as tile
from concourse.bass_utils import run_bass_kernel_spmd

B, NQ, NK, D, H = 2, 2048, 2048, 1024, 16
DH = D // H  # 64
N_CORES = 8
HPC = H // (N_CORES // B)  # heads per core = 4
JW = HPC * DH  # per-core projection width = 256
NKT = NK // 128  # 16 nk tiles
NCH = 4  # nq chunks
CHW = NQ // NCH  # 512
DT = 8  # contraction d-tiles

f32 = mybir.dt.float32
f32r = mybir.dt.float32r
bf16 = mybir.dt.bfloat16
u8 = mybir.dt.uint8


def _build():
    nc = bacc.Bacc(
        "TRN2", target_bir_lowering=False, debug=False, num_devices=N_CORES
    )

    # x tensors in partition-major chunk layout: X[p, ch, d, n] =
    # x[ch*CHW + n, d*128 + p] -- one contiguous 8KB run per partition
    # per chunk, so a chunk is a single 128-descriptor dma_start.
    qTd = nc.dram_tensor("qT", [128, NCH, DT, CHW], bf16, kind="ExternalInput")
    kTd = nc.dram_tensor("kT", [128, NCH, DT, CHW], bf16, kind="ExternalInput")
    vTd = nc.dram_tensor("vT", [128, NCH, DT, CHW], bf16, kind="ExternalInput")
    # mask: M[p, t, n] = mask[n, t*128 + p]
    maskd = nc.dram_tensor("maskT", [128, NKT, NQ], u8, kind="ExternalInput")
    # weights: W[p, d, j] = w[d*128 + p, j]
    wqd = nc.dram_tensor("wqT", [128, DT, JW], bf16, kind="ExternalInput")
    wkd = nc.dram_tensor("wkT", [128, DT, JW], bf16, kind="ExternalInput")
    wvd = nc.dram_tensor("wvT", [128, DT, JW], bf16, kind="ExternalInput")
    bqd = nc.dram_tensor("bq", [128, 2], f32, kind="ExternalInput")
    bkd = nc.dram_tensor("bk", [128, 2], f32, kind="ExternalInput")
    # bvp[p, pair] = bv[128*pair + p]
    bvpd = nc.dram_tensor("bvp", [128, 2], f32, kind="ExternalInput")
    # ones2[64p, 0:64] = 1, ones2[64p + 32, 64:128] = 1
    onesd = nc.dram_tensor("ones2", [128, 128], f32r, kind="ExternalInput")
    onespd = nc.dram_tensor("onesp", [128, 32], bf16, kind="ExternalInput")
    o = nc.dram_tensor("o", [2 * 128, NQ], bf16, kind="ExternalOutput")

    with tile.TileContext(nc) as tc:
        with (
            tc.tile_pool(name="consts", bufs=1) as consts,
            tc.tile_pool(name="kst", bufs=3) as kst,
            tc.tile_pool(name="qst", bufs=2) as qst,
            tc.tile_pool(name="vst", bufs=3) as vst,
            tc.tile_pool(name="m8pool", bufs=8) as m8pool,
            tc.tile_pool(name="mbpool", bufs=7) as mbpool,
            tc.tile_pool(name="projout", bufs=1) as projout,
            tc.tile_pool(name="ppool", bufs=16) as ppool,
            tc.tile_pool(name="rspool", bufs=1) as rspool,
            tc.tile_pool(name="outsb", bufs=2) as outsb,
            tc.tile_pool(name="sps", bufs=2, space="PSUM") as sps,
            tc.tile_pool(name="pvps", bufs=2, space="PSUM") as pvps,
            tc.tile_pool(name="rsps", bufs=2, space="PSUM") as rsps,
        ):
            def dma_w(name, dram):
                t = consts.tile([128, DT, JW], bf16, tag=f"w{name}", name="w")
                nc.sync.dma_start(t, dram[:])
                return t

            def dma_x(src, ch, pool, tag, split=False):
                x = pool.tile([128, DT, CHW], bf16, tag=tag, name=tag)
                if split:
                    # two halves so the d 0-3 matmuls can start earlier
                    nc.sync.dma_start(x[:, 0:4], src[:, ch, 0:4])
                    nc.sync.dma_start(x[:, 4:8], src[:, ch, 4:8])
                else:
                    nc.sync.dma_start(x, src[:, ch])
                return x

            def dma_m(g):
                """Mask tiles 2g, 2g+1. On the sync queue, placed in
                consumption order: the DMA engines drain mostly FIFO, so a
                parallel-queue mask would overtake the critical k/q path."""
                mt8 = m8pool.tile([128, 2, NQ], u8, tag="m8", name="m8")
                nc.sync.dma_start(mt8, maskd[:, 2 * g : 2 * g + 2, :])
                return mt8

            # ---- input DMAs, emitted in consumption order ----
            bq_sb = consts.tile([128, 2], f32, tag="bq")
            nc.sync.dma_start(bq_sb, bqd[:])
            bk_sb = consts.tile([128, 2], f32, tag="bk")
            nc.sync.dma_start(bk_sb, bkd[:])
            onesp_sb = consts.tile([128, 32], bf16, tag="onesp")
            nc.sync.dma_start(onesp_sb, onespd[:])
            w_k = dma_w("k", wkd)
            k_x = {0: dma_x(kTd, 0, kst, "kx", split=True)}
            w_q = dma_w("q", wqd)
            q_x = {0: dma_x(qTd, 0, qst, "qx", split=True)}
            m8 = [dma_m(0)]
            k_x[1] = dma_x(kTd, 1, kst, "kx", split=True)
            m8.append(dma_m(1))
            w_v = dma_w("v", wvd)
            v_x = {0: dma_x(vTd, 0, vst, "vx", split=True)}
            m8.append(dma_m(2))
            v_x[1] = dma_x(vTd, 1, vst, "vx")
            k_x[2] = dma_x(kTd, 2, kst, "kx")
            m8.append(dma_m(3))
            v_x[2] = dma_x(vTd, 2, vst, "vx")
            m8.append(dma_m(4))
            m8.append(dma_m(5))
            k_x[3] = dma_x(kTd, 3, kst, "kx")
            v_x[3] = dma_x(vTd, 3, vst, "vx")
            m8.append(dma_m(6))
            m8.append(dma_m(7))
            bvp_sb = consts.tile([128, 2], f32, tag="bvp")
            nc.sync.dma_start(bvp_sb, bvpd[:])
            ones_sb = consts.tile([128, 128], f32r, tag="ones")
            nc.sync.dma_start(ones_sb, onesd[:])
            q_x[1] = dma_x(qTd, 1, qst, "qx")
            q_x[2] = dma_x(qTd, 2, qst, "qx")
            q_x[3] = dma_x(qTd, 3, qst, "qx")

            # ---- projection outputs (split per chunk for clean deps) ----
            ktTs = [
                projout.tile([128, 2, CHW], bf16, tag=f"ktT{c}", name="ktT")
                for c in range(NCH)
            ]
            qtTs = [
                projout.tile([128, 2, CHW], bf16, tag=f"qtT{c}", name="qtT")
                for c in range(NCH)
            ]
            # vts[g][:, a, :] = vt for nk-tile 2g+a
            vts = [
                projout.tile([128, 2, JW], bf16, tag=f"vt{g}", name="vt")
                for g in range(NKT // 2)
            ]

            def proj_qk_sps(w, xs, dst, bias):
                """q/k projection chunk through one 2-bank sps tile:
                m0 -> cols 0:CHW, m1 -> cols CHW:2CHW."""
                ps = sps.tile([128, 2 * CHW], f32, tag="s", name="pps")
                for d in range(DT):
                    for m in range(2):
                        nc.tensor.matmul(
                            ps[:, m * CHW : (m + 1) * CHW],
                            w[:, d, m * 128 : (m + 1) * 128],
                            xs[:, d],
                            start=(d == 0),
                            stop=(d == DT - 1),
                        )
                for m in range(2):
                    nc.vector.tensor_scalar_add(
                        dst[:, m, :],
                        ps[:, m * CHW : (m + 1) * CHW],
                        bias[:, m : m + 1],
                    )

            def proj_qk_m(w, xs, dst, bias, m, pool, nm):
                """One m-half of a q/k projection chunk through a single
                [128, CHW] psum tile from `pool`."""
                ps = pool.tile([128, CHW], f32, tag=nm, name="pps")
                for d in range(DT):
                    nc.tensor.matmul(
                        ps,
                        w[:, d, m * 128 : (m + 1) * 128],
                        xs[:, d],
                        start=(d == 0),
                        stop=(d == DT - 1),
                    )
                nc.vector.tensor_scalar_add(
                    dst[:, m, :], ps, bias[:, m : m + 1]
                )

            def proj_qk_m_part(w, xs, dst, bias, m, part, state):
                """proj_qk_m split into two 4-matmul halves so a steady
                chunk's next-q projection never spikes one iteration's PE
                load."""
                if part == 0:
                    state[m] = rsps.tile([128, CHW], f32, tag="rspst", name="pps")
                ps = state[m]
                for d in (range(0, 4) if part == 0 else range(4, DT)):
                    nc.tensor.matmul(
                        ps,
                        w[:, d, m * 128 : (m + 1) * 128],
                        xs[:, d],
                        start=(d == 0),
                        stop=(d == DT - 1),
                    )
                if part == 1:
                    nc.vector.tensor_scalar_add(
                        dst[:, m, :], ps, bias[:, m : m + 1]
                    )

            def vproj_pair(g):
                """vt for nk-tiles 2g, 2g+1 through the rsps utility slot."""
                ps = rsps.tile([128, CHW], f32, tag="rspst", name="vps")
                ps2 = ps.rearrange("p (a j) -> p a j", a=2)
                for d in range(DT):
                    for a in range(2):
                        t = 2 * g + a
                        ch, nn_ = divmod(t, 4)
                        # start=True clears the WHOLE bank (all 512 cols) in
                        # the written partitions, so only the very first
                        # matmul of the packed pair may set it; the a=1
                        # group's first matmul overwrites where has_written
                        # is clear (bank-wide clear reset its bits too).
                        nc.tensor.matmul(
                            ps2[:, a],
                            v_x[ch][:, d, nn_ * 128 : (nn_ + 1) * 128],
                            w_v[:, d, :],
                            start=(d == 0 and a == 0),
                            stop=(d == DT - 1),
                        )
                nc.vector.tensor_copy(vts[g][:], ps2)

            # ---- attention ----
            def scores_group(pair, t, ch, p_tiles):
                sp = sps.tile([128, 2 * CHW], f32, tag="s", name="sp")
                kc, tt = ktTs[t // 4], t % 4
                for hh in range(2):
                    nc.tensor.matmul(
                        sp[:, hh * CHW : (hh + 1) * CHW],
                        kc[
                            64 * hh : 64 * (hh + 1),
                            pair,
                            tt * 128 : (tt + 1) * 128,
                        ],
                        qtTs[ch][64 * hh : 64 * (hh + 1), pair, :],
                        start=True,
                        stop=True,
                    )
                p = ppool.tile([128, 2 * CHW], bf16, tag="p", name="p")
                nc.scalar.activation(
                    out=p,
                    in_=sp,
                    func=mybir.ActivationFunctionType.Exp,
                    scale=1.0 / 32.0,
                )
                if pair == 0:
                    mb = mbpool.tile([128, CHW], bf16, tag="mb", name="mb")
                    # SWDGE cast DMA u8 -> bf16 (frees GpSimd compute)
                    nc.gpsimd.dma_start(
                        mb, m8[t // 2][:, t % 2, ch * CHW : (ch + 1) * CHW]
                    )
                    p_tiles[("mb", t)] = mb
                p_tiles[(pair, t)] = p

            def mask_mul(t, p_tiles):
                # deferred from scores_group so utility-phase drains don't
                # queue behind a burst of muls on the in-order DVE queue
                mb = p_tiles[("mb", t)]
                for pair in range(2):
                    p3 = p_tiles[(pair, t)].rearrange("p (h c) -> p h c", h=2)
                    nc.vector.tensor_mul(
                        p3,
                        p3,
                        mb.rearrange("p (a c) -> p a c", a=1).to_broadcast(
                            (128, 2, CHW)
                        ),
                    )

            def pv_t(t, p_tiles, pv_ps, rs_ps):
                st, sp_ = t == 0, t == NKT - 1
                g, a = divmod(t, 2)
                for pair in range(2):
                    p = p_tiles[(pair, t)]
                    for hh in range(2):
                        nc.tensor.matmul(
                            pv_ps[pair][64 * hh : 64 * (hh + 1), :],
                            vts[g][
                                :,
                                a,
                                128 * pair + 64 * hh : 128 * pair
                                + 64 * (hh + 1),
                            ],
                            p[:, hh * CHW : (hh + 1) * CHW],
                            start=st,
                            stop=sp_,
                            tile_position=(0, 64 * hh),
                        )
                for pair in range(2):
                    p = p_tiles[(pair, t)]
                    for hh in range(2):
                        hg = 2 * pair + hh
                        nc.tensor.matmul(
                            rs_ps[32 * hg : 32 * hg + 32, :],
                            onesp_sb[:, 0:32],
                            p[:, hh * CHW : (hh + 1) * CHW],
                            start=st,
                            stop=sp_,
                            tile_position=(0, 32 * hg),
                        )

            def chunk_tail(cs, pv_ps, rs_ps):
                # rowsum -> +eps -> reciprocal (all 128 rows valid: the M=32
                # rowsum matmuls wrote 32 identical rows per head). The whole
                # tail runs on DVE + GpSimd: no PE matmuls and no ACT ops, so
                # it never blocks next-chunk scores or the exp stream.
                rs_sb = rspool.tile([128, CHW], f32, tag="rssb", name="rssb")
                nc.vector.tensor_scalar_add(rs_sb, rs_ps, 1e-6)
                rc_sb = rspool.tile([128, CHW], f32, tag="rcsb", name="rcsb")
                nc.vector.reciprocal_approx_fast(out=rc_sb, in_=rs_sb)
                # bit-identical copy to f32r so the rank-1 broadcast matmul
                # streams fast (plain-f32 moving is 4x slower on the PE)
                rc_r = rspool.tile([128, CHW], f32r, tag="rcr", name="rcr")
                nc.vector.tensor_copy(rc_r, rc_sb)
                for pair in range(2):
                    # rb = broadcast of 1/(rs+eps) to the pair's 128 rows
                    rb = rsps.tile([128, CHW], f32, tag="rspst", name="rb")
                    nc.tensor.matmul(
                        rb,
                        ones_sb[64 * pair : 64 * (pair + 1), :],
                        rc_r[64 * pair : 64 * (pair + 1), :],
                        start=True,
                        stop=True,
                    )
                    rb_sb = outsb.tile([128, CHW], f32, tag="rbsb", name="rbsb")
                    nc.vector.tensor_copy(rb_sb, rb)
                    osb = outsb.tile([128, CHW], bf16, tag="o", name="osb")
                    nc.vector.tensor_mul(osb, pv_ps[pair], rb_sb)
                    # out = pv/rs + bv  (the V-bias passes straight through
                    # the softmax normalization)
                    nc.vector.tensor_scalar_add(
                        osb, osb, bvp_sb[:, pair : pair + 1]
                    )
                    nc.gpsimd.dma_start(
                        o[128 * pair : 128 * (pair + 1), cs], osb
                    )

            def new_pv_tiles():
                pv_ps = [
                    pvps.tile([128, CHW], f32, tag="pvpst", name=f"pv{i}")
                    for i in range(2)
                ]
                rs_ps = rsps.tile([128, CHW], f32, tag="rspst", name="rsps_t")
                return pv_ps, rs_ps

            # ---- pre-phase: k-proj chunks 0+1, q-proj chunk0 ----
            proj_qk_sps(w_k, k_x[0], ktTs[0], bk_sb)
            proj_qk_sps(w_q, q_x[0], qtTs[0], bq_sb)

            # chunk 0 utility-phase schedule (kc1 through the pvps slots
            # before PV claims them; everything else through the rsps
            # rotating slot). Keyed by iteration; placed so each phase's
            # inputs (DMA) land just before and its output is ready just
            # before its first consumer. Emitted AFTER that iteration's
            # scores so the exps fire first.
            util0 = {
                1: [lambda: proj_qk_m(w_k, k_x[1], ktTs[1], bk_sb, 0, pvps, "pvpst")],
                2: [lambda: proj_qk_m(w_k, k_x[1], ktTs[1], bk_sb, 1, pvps, "pvpst")],
                3: [lambda: vproj_pair(0)],
                4: [lambda: vproj_pair(1)],
                5: [lambda: vproj_pair(2)],
                6: [lambda: proj_qk_m(w_k, k_x[2], ktTs[2], bk_sb, 0, rsps, "rspst")],
                7: [lambda: proj_qk_m(w_k, k_x[2], ktTs[2], bk_sb, 1, rsps, "rspst")],
                8: [lambda: vproj_pair(3)],
                9: [lambda: vproj_pair(4)],
                10: [lambda: proj_qk_m(w_k, k_x[3], ktTs[3], bk_sb, 0, rsps, "rspst")],
                11: [lambda: proj_qk_m(w_k, k_x[3], ktTs[3], bk_sb, 1, rsps, "rspst")],
                12: [lambda: vproj_pair(5)],
                13: [lambda: vproj_pair(6)],
                14: [lambda: vproj_pair(7)],
            }

            # All chunks fully interleaved. The last LAG PV steps + tail of
            # chunk c are carried into chunk c+1's first iterations so the
            # ACT stream never waits for a PE drain at a chunk boundary.
            # Chunk c+1's q-projection runs mid-chunk-c through the rsps
            # slot (free after the carried tail releases it).
            LAG = 3
            pending = []
            for ch in range(NCH):
                cs = slice(ch * CHW, (ch + 1) * CHW)
                p_tiles = {}
                qp_state = {}
                pv_ps, rs_ps = None, None
                for t in range(NKT):
                    if (ch == 0 and t == 3) or (ch > 0 and t == 0):
                        pv_ps, rs_ps = new_pv_tiles()
                    for pair in range(2):
                        scores_group(pair, t, ch, p_tiles)
                    if ch == 0 and t in util0:
                        for fn in util0[t]:
                            fn()
                    if pending:
                        pending.pop(0)()
                    if t >= 2:
                        mask_mul(t - 2, p_tiles)
                    if t >= LAG:
                        pv_t(t - LAG, p_tiles, pv_ps, rs_ps)
                    if ch + 1 < NCH:
                        if ch == 0:
                            if t == 14:
                                proj_qk_m(w_q, q_x[1], qtTs[1], bq_sb, 0,
                                          rsps, "rspst")
                            elif t == 15:
                                proj_qk_m(w_q, q_x[1], qtTs[1], bq_sb, 1,
                                          rsps, "rspst")
                        elif 11 <= t <= 14:
                            m, part = divmod(t - 11, 2)
                            proj_qk_m_part(
                                w_q, q_x[ch + 1], qtTs[ch + 1], bq_sb,
                                m, part, qp_state,
                            )
                # muls for the last two tiles before their carried PVs
                mask_mul(NKT - 2, p_tiles)
                mask_mul(NKT - 1, p_tiles)

                def _carry(tt, p_tiles=p_tiles, pv_ps=pv_ps, rs_ps=rs_ps):
                    pv_t(tt, p_tiles, pv_ps, rs_ps)

                def _tail(cs=cs, pv_ps=pv_ps, rs_ps=rs_ps):
                    chunk_tail(cs, pv_ps, rs_ps)

                pending = [
                    lambda f=_carry: f(NKT - 3),
                    lambda f=_carry: f(NKT - 2),
                    lambda f=_carry, g=_tail: (f(NKT - 1), g()),
                ]
            for fn in pending:
                fn()

    nc.compile()
    return nc


_NC = None


def _get_nc():
    global _NC
    if _NC is None:
        _NC = _build()
    return _NC


def _shard(inputs):
    import ml_dtypes

    bfl = ml_dtypes.bfloat16
    q, k, v = inputs["q"], inputs["k"], inputs["v"]
    mask = inputs["mask"]
    Wq, bq, Wk, bk, Wv, bv = (
        inputs[n] for n in ("Wq", "bq", "Wk", "bk", "Wv", "bv")
    )

    def xfmt(x):
        # [N, D] -> [128, NCH, DT, CHW]: X[p, ch, d, n] = x[ch*CHW+n, 128d+p]
        return np.ascontiguousarray(
            np.asarray(x)
            .reshape(NCH, CHW, DT, 128)
            .transpose(3, 0, 2, 1)
            .astype(bfl)
        )

    def wfmt(w, j0):
        # [D, D] -> [128, DT, JW]: W[p, d, j] = w[j0+j, 128d+p]
        return np.ascontiguousarray(
            np.asarray(w)[j0 : j0 + JW, :].T.reshape(DT, 128, JW)
            .transpose(1, 0, 2)
            .astype(bfl)
        )

    qX = [xfmt(q[b]) for b in range(B)]
    kX = [xfmt(k[b]) for b in range(B)]
    vX = [xfmt(v[b]) for b in range(B)]
    # mask [NQ, NK] bool -> [128, NKT, NQ] u8: M[p, t, n] = mask[n, 128t+p]
    mX = [
        np.ascontiguousarray(
            np.asarray(mask[b]).T.reshape(NKT, 128, NQ).transpose(1, 0, 2)
        ).view(np.uint8)
        for b in range(B)
    ]
    onesp = np.ones((128, 32), bfl)
    ones2 = np.zeros((128, 128), np.float32)
    for p in range(2):
        ones2[64 * p, 0:64] = 1.0
        ones2[64 * p + 32, 64:128] = 1.0
    in_maps = []
    for c in range(N_CORES):
        b, jg = divmod(c, N_CORES // B)
        j0 = jg * JW
        bvp = np.ascontiguousarray(
            np.asarray(bv, np.float32)[j0 : j0 + JW].reshape(2, 128).T
        )
        in_maps.append(
            {
                "qT": qX[b],
                "kT": kX[b],
                "vT": vX[b],
                "maskT": mX[b],
                "wqT": wfmt(Wq, j0),
                "wkT": wfmt(Wk, j0),
                "wvT": wfmt(Wv, j0),
                "bq": np.ascontiguousarray(
                    np.asarray(bq, np.float32)[j0 : j0 + JW].reshape(2, 128).T
                ),
                "bk": np.ascontiguousarray(
                    np.asarray(bk, np.float32)[j0 : j0 + JW].reshape(2, 128).T
                ),
                "bvp": bvp,
                "ones2": ones2,
                "onesp": onesp,
            }
        )
    return in_maps


LAST_RESULT = None


def kernel(**inputs) -> np.ndarray:
    global LAST_RESULT
    nc = _get_nc()
    in_maps = _shard(inputs)
    trace = bool(int(os.environ.get("KTRACE", "0")))
    res = run_bass_kernel_spmd(
        nc,
        in_maps,
        core_ids=list(range(N_CORES)),
        trace=trace,
        trace_cores=[0] if trace else None,
    )
    LAST_RESULT = res
    out = np.empty((B, NQ, D), np.float32)
    for c in range(N_CORES):
        b, jg = divmod(c, N_CORES // B)
        j0 = jg * JW
        oc = res.results[c]["o"].astype(np.float32)  # [256, NQ] pair-major
        out[b, :, j0 : j0 + JW] = (
            oc.reshape(2, 2, DH, NQ).transpose(3, 0, 1, 2).reshape(NQ, JW)
        )
    return out


if __name__ == "__main__":
    if os.environ.get("KBUILD_ONLY"):
        import tempfile

        from concourse.bass_utils import compile_bass_kernel

        nc = _build()
        with tempfile.TemporaryDirectory() as td:
            compile_bass_kernel(nc, td)
        print("BUILD+COMPILE OK")
